# revision 19
# baseline (speedup 1.0000x reference)
"""Trainium2 Bass kernel for the LSTM decoder — v5: four interleaved
quarter-batch recurrences.

Per core (256 batch rows) the batch is split into four independent 64-column
recurrences phase-shifted by a quarter step.  The Activation engine is the
busiest engine; with four streams its work arrives as (tanh(q), sigma(q+1))
pairs of ~1436ns, one pair per quarter-phase, so ACT packs to ~100% and sets
the period at its busy time instead of the serial sigma->c-chain->tanh loop
that limited the two-half schedule.

- PSUM per quarter: one [128, 1024] region (2 banks): bank0 = [i, g] tiles,
  bank1 = [f, o] tiles, 16 tiles of [128 gate rows, 64 batch].
- One merged sigmoid ACT [128, 1024] per quarter-step covers all four gate
  classes (tanh(g) = 2*sigmoid(2g)-1 with the g-rows of W pre-doubled);
  one [128, 256] tanh for the c update.
- DVE per quarter-step: gtil = 2*u_g-1 (4x tensor_scalar), t1 = u_i*gtil,
  t2 = u_f*c, c' = t1+t2, then h8 = u_o*tanh(c') written as two [128, 128]
  fp8 chunks (kc01 matmuls of the next step start after the first chunk)
  and h16 (fp16, for y) as one [128, 256] mul.
- The merged sigmoid's scale operand is a [128,1] SBUF tile holding 1/SP,
  rewritten after the PREVIOUS quarter's c-update: this orders sigma(q)
  after ct(q-1) in the ACT queue so big sigmoids never preempt a ready tanh.
- fp8 (e4m3) DoubleRow matmuls: the two K-slots carry the (hi, lo) split of
  the merged W = W_ih + W_hh (valid since output h feeds back as next
  input); moving operand is the fp8 h chunk broadcast into both slots.
- y_t = W_d @ h_t (fp16) lands in the o-s0 PSUM tile after the merged
  sigmoid reads it; copied out by the idle Pool engine; scaled and
  bias-added on the host.
"""

import numpy as np
import ml_dtypes
from contextlib import ExitStack

import concourse.bacc as bacc
import concourse.mybir as mybir
from concourse import tile
from concourse.bass_utils import run_bass_kernel_spmd

fp32 = mybir.dt.float32
fp16 = mybir.dt.float16
fp8 = mybir.dt.float8e4
F8 = ml_dtypes.float8_e4m3fn
AF = mybir.ActivationFunctionType
ALU = mybir.AluOpType
DR = mybir.MatmulPerfMode.DoubleRow

P = 128
B = 256          # batch rows per core
NQ = 4           # interleaved recurrences per core
QW = 64          # quarter-batch width
HC = 4           # hidden chunks of 128
NT = 16          # gate tiles per quarter
PH = 32
NCORES = 8
SP = 1024.0      # weight/bias scale (keeps all fp8 <= 240: IEEE-e4m3 safe)

# gate-class order in PSUM banks: [i, g | f, o]; W row bases (torch i,f,g,o)
RB = [0, 1024, 512, 1536]

_CACHE = {}


def _build():
    nc = bacc.Bacc("TRN2", target_bir_lowering=False, debug=False,
                   num_devices=NCORES)

    ws_d = nc.dram_tensor("ws", [P, HC, 2, NT, P], fp8, kind="ExternalInput")
    wih_d = nc.dram_tensor("wih", [P, HC, 2, NT, P], fp8, kind="ExternalInput")
    bs_d = nc.dram_tensor("bs", [1, 2, NT, P], fp8, kind="ExternalInput")
    wd_d = nc.dram_tensor("wd", [P, HC, 2], fp16, kind="ExternalInput")
    zhi_d = nc.dram_tensor("zhi", [P, HC, B], fp8, kind="ExternalInput")
    zlo_d = nc.dram_tensor("zlo", [P, HC, B], fp8, kind="ExternalInput")
    ones_d = nc.dram_tensor("ones", [1, QW], fp8, kind="ExternalInput")
    y_d = nc.dram_tensor("y", [2, PH * B], fp32, kind="ExternalOutput")

    with tile.TileContext(nc) as tc:
        with ExitStack() as ctx:
            const = ctx.enter_context(tc.tile_pool(name="const", bufs=1))
            state = ctx.enter_context(tc.tile_pool(name="state", bufs=1))
            pp = ctx.enter_context(tc.tile_pool(name="pp", bufs=1,
                                                space="PSUM"))

            wih = const.tile([P, HC, 2, NT, P], fp8)
            for kc in range(HC):
                nc.sync.dma_start(wih[:, kc], wih_d[:, kc])
            zhi = const.tile([P, HC, B], fp8)
            zlo = const.tile([P, HC, B], fp8)
            nc.sync.dma_start(zhi[:], zhi_d[:])
            nc.sync.dma_start(zlo[:], zlo_d[:])
            bs = const.tile([1, 2, NT, P], fp8)
            nc.sync.dma_start(bs[:], bs_d[:])
            ones = const.tile([1, QW], fp8)
            nc.sync.dma_start(ones[:], ones_d[:])
            wd = const.tile([P, HC, 2], fp16)
            nc.sync.dma_start(wd[:], wd_d[:])
            ws = const.tile([P, HC, 2, NT, P], fp8)
            for kc in range(HC):
                nc.sync.dma_start(ws[:, kc], ws_d[:, kc])

            pH = [pp.tile([P, 1024], fp32, tag=f"p{q}", name=f"p{q}")
                  for q in range(NQ)]
            u = [state.tile([P, 1024], fp16, tag=f"u{q}", name=f"u{q}")
                 for q in range(NQ)]
            ct = [state.tile([P, 256], fp16, tag=f"c{q}", name=f"c{q}")
                  for q in range(NQ)]
            tct = [state.tile([P, 256], fp16, tag=f"tc{q}", name=f"tc{q}")
                   for q in range(NQ)]
            gt = [state.tile([P, 256], fp16, tag=f"gt{q}", name=f"gt{q}")
                  for q in range(NQ)]
            t1 = [state.tile([P, 256], fp16, tag=f"t1{q}", name=f"t1{q}")
                  for q in range(NQ)]
            t2 = [state.tile([P, 256], fp16, tag=f"t2{q}", name=f"t2{q}")
                  for q in range(NQ)]
            h8 = [[state.tile([P, HC * QW], fp8, tag=f"h8{q}b{b}",
                              name=f"h8{q}b{b}") for b in range(2)]
                  for q in range(NQ)]
            h16 = [[state.tile([P, HC * QW], fp16, tag=f"h16{q}b{b}",
                               name=f"h16{q}b{b}") for b in range(2)]
                   for q in range(NQ)]
            sc = [state.tile([P, 1], fp32, tag=f"sc{q}", name=f"sc{q}")
                  for q in range(NQ)]
            y_sb = const.tile([2, PH * B], fp32)
            ones_b = ones[:].unsqueeze(1).broadcast_to([1, 2, QW])

            import os
            PACE = os.environ.get("KPACE", "u")    # "none" | "u"
            T2ENG = os.environ.get("KT2", "dve")   # "dve" | "pool"

            def mov(src_ap):
                return src_ap.unsqueeze(1).broadcast_to([P, 2, QW])

            def out_ap(q, tau):
                return pH[q][:, QW * tau:QW * (tau + 1)]

            def bias_mm(q, beta, s):
                tau = 4 * beta + s
                nc.tensor.matmul(out_ap(q, tau),
                                 bs[0:1, :, tau, :], ones_b,
                                 start=(tau in (0, 8)),
                                 stop=False, perf_mode=DR)

            def kg_mm(q, tau, kc, W, src_ap, stop):
                nc.tensor.matmul(out_ap(q, tau),
                                 W[:, kc, :, tau, :], mov(src_ap),
                                 start=False, stop=stop, perf_mode=DR)

            def hsrc(q, t, kc):
                return h8[q][t % 2][:, kc * QW:(kc + 1) * QW]

            def bias_all(q):
                for beta in range(4):
                    for s in range(HC):
                        bias_mm(q, beta, s)

            def kg(q, t):
                # kc01 first (gated by the first h8 chunk), then kc23
                for kcp in ((0, 1), (2, 3)):
                    for kc in kcp:
                        for tau in range(NT):
                            kg_mm(q, tau, kc, ws, hsrc(q, t - 1, kc),
                                  stop=(kc == 3 and tau in (7, 15)))

            def y_mm(q, t):
                # fp16 matmul from the fp16 h copy; lands in the o-s0 PSUM
                # tile after the merged sigmoid reads it
                out = pH[q][0:2, 768:768 + QW]
                for kc in range(HC):
                    nc.tensor.matmul(out, wd[:, kc, :],
                                     h16[q][t % 2][:, kc * QW:(kc + 1) * QW],
                                     start=(kc == 0), stop=(kc == 3))

            def y_copy(q, t):
                nc.gpsimd.tensor_copy(y_sb[:, B * t + QW * q:
                                           B * t + QW * (q + 1)],
                                      pH[q][0:2, 768:768 + QW])

            def step0_mm(q):
                # x = z (two-term); f-gate z-matmuls skipped (c0 = 0), but
                # its bias still lands so sigma reads finite values
                for beta in range(4):
                    for s in range(HC):
                        bias_mm(q, beta, s)
                for kc in range(HC):
                    zh = zhi[:, kc, QW * q:QW * (q + 1)]
                    zl = zlo[:, kc, QW * q:QW * (q + 1)]
                    for beta in (0, 1, 3):
                        for s in range(HC):
                            tau = 4 * beta + s
                            kg_mm(q, tau, kc, wih, zh, stop=False)
                            kg_mm(q, tau, kc, wih, zl,
                                  stop=(kc == 3 and tau in (7, 15)))

            def chain(q, t):
                nc.scalar.activation(u[q][:], pH[q][:], AF.Sigmoid,
                                     scale=(sc[q][:] if PACE == "u"
                                            else 1.0 / SP))
                if PACE == "u":
                    # pace the next quarter's sigmoid: its scale becomes
                    # ready only once this sigmoid has completed
                    nc.vector.tensor_scalar(sc[(q + 1) % NQ][:],
                                            u[q][:, 0:1], 0.0, 1.0 / SP,
                                            ALU.mult, ALU.add)
                # DVE c-chain
                nc.vector.tensor_scalar(gt[q][:], u[q][:, 256:512], 2.0, 1.0,
                                        ALU.mult, ALU.subtract)
                if t == 0:
                    nc.vector.tensor_mul(ct[q][:], u[q][:, 0:256], gt[q][:])
                else:
                    t2e = nc.gpsimd if T2ENG == "pool" else nc.vector
                    t2e.tensor_mul(t2[q][:], u[q][:, 512:768], ct[q][:])
                    nc.vector.tensor_mul(t1[q][:], u[q][:, 0:256], gt[q][:])
                    nc.vector.tensor_add(ct[q][:], t1[q][:], t2[q][:])
                nc.scalar.activation(tct[q][:], ct[q][:], AF.Tanh)
                hb = h8[q][t % 2]
                nc.vector.tensor_mul(hb[:, 0:128], u[q][:, 768:896],
                                     tct[q][:, 0:128])
                nc.vector.tensor_mul(hb[:, 128:256], u[q][:, 896:1024],
                                     tct[q][:, 128:256])
                # h16 feeds y_mm two steps later: plenty of slack on Pool
                nc.gpsimd.tensor_mul(h16[q][t % 2][:], u[q][:, 768:1024],
                                     tct[q][:])

            # --- step 0 ---
            if PACE == "u":
                for q in range(NQ):
                    nc.vector.tensor_scalar(sc[q][:], zhi[:, 0, 0:1], 0.0,
                                            1.0 / SP, ALU.mult, ALU.add)
            for q in range(NQ):
                step0_mm(q)
                chain(q, 0)

            # --- steady steps ---
            for t in range(1, PH):
                for q in range(NQ):
                    if t >= 2:
                        y_mm(q, t - 2)
                        y_copy(q, t - 2)
                    bias_all(q)
                    kg(q, t)
                    chain(q, t)

            # --- drain the y tail ---
            for t in (PH - 2, PH - 1):
                for q in range(NQ):
                    y_mm(q, t)
                    y_copy(q, t)
            nc.sync.dma_start(y_d[:], y_sb[:])
    nc.compile()
    return nc


def _get_nc():
    if "nc" not in _CACHE:
        _CACHE["nc"] = _build()
    return _CACHE["nc"]


def _enc8(x):
    return np.asarray(F8(np.asarray(x, np.float32)))


def _prep_inputs(z, W_ih, W_hh, b_ih, b_hh, W_d):
    z2 = np.asarray(z, np.float32).reshape(2048, 512)
    W_ih = np.asarray(W_ih, np.float32)
    W_sum = W_ih + np.asarray(W_hh, np.float32)
    bias = (np.asarray(b_ih, np.float32) + np.asarray(b_hh, np.float32))

    def fold_w(W):
        W2 = W * SP
        W2[1024:1536] *= 2.0
        return W2

    W2 = fold_w(W_sum)
    Wih2 = fold_w(W_ih)
    Bp = bias * SP
    Bp[1024:1536] *= 2.0

    # tile tau = 4*beta + s -> W rows RB[beta] + 128*s
    rows = np.empty((NT, P), np.int64)
    for beta in range(4):
        for s in range(HC):
            rows[4 * beta + s] = RB[beta] + 128 * s + np.arange(P)

    def to_ws(W2f):
        hi = _enc8(W2f).astype(np.float32)
        lo = _enc8(W2f - hi).astype(np.float32)

        def lay(Wq):
            a = Wq[rows]                                      # [16,128,512]
            a = a.reshape(NT, P, HC, P).transpose(3, 2, 0, 1)  # [p,kc,tau,m]
            return a
        out = np.stack([lay(hi), lay(lo)], axis=2)             # [p,kc,2,tau,m]
        return np.ascontiguousarray(_enc8(out))

    ws = to_ws(W2)
    wih = to_ws(Wih2)

    bhi = _enc8(Bp).astype(np.float32)
    blo = _enc8(Bp - bhi).astype(np.float32)
    bs = np.stack([bhi[rows], blo[rows]], axis=0)
    bs = np.ascontiguousarray(_enc8(bs[None]))                 # [1,2,16,128]

    Wd2 = np.asarray(W_d, np.float32) * SP
    wd = np.ascontiguousarray(
        Wd2.T.reshape(HC, P, 2).transpose(1, 0, 2)).astype(np.float16)

    ones = _enc8(np.ones((1, QW), np.float32))

    in_maps = []
    for cix in range(NCORES):
        zc = z2[cix * B:(cix + 1) * B].T                       # [512, 256]
        zhi = _enc8(zc).astype(np.float32)
        zlo = _enc8(zc - zhi).astype(np.float32)
        zhi = zhi.reshape(HC, P, B).transpose(1, 0, 2)
        zlo = zlo.reshape(HC, P, B).transpose(1, 0, 2)
        in_maps.append({
            "ws": ws, "wih": wih, "bs": bs, "wd": wd,
            "zhi": np.ascontiguousarray(_enc8(zhi)),
            "zlo": np.ascontiguousarray(_enc8(zlo)),
            "ones": ones,
        })
    return in_maps


def run(inputs, trace=False, **kw):
    nc = _get_nc()
    in_maps = _prep_inputs(inputs["z"], inputs["W_ih"], inputs["W_hh"],
                           inputs["b_ih"], inputs["b_hh"], inputs["W_d"])
    res = run_bass_kernel_spmd(nc, in_maps, core_ids=list(range(NCORES)),
                               trace=trace, **kw)
    b_d = np.asarray(inputs["b_d"], np.float32)
    outs = []
    for cix in range(NCORES):
        arr = res.results[cix]["y"] / SP                       # [2, PH*B]
        outs.append(arr.reshape(2, PH, B).transpose(2, 1, 0))
    y = np.concatenate(outs, axis=0) + b_d[None, None, :]
    return np.ascontiguousarray(y, dtype=np.float32), res


def kernel(**inputs):
    y, _ = run(inputs, trace=False)
    return y
